# revision 1
# baseline (speedup 1.0000x reference)
"""Trainium2 Bass kernel for GQA causal self-attention (B=4, S=2048, D=1024,
16 query heads / 4 kv heads, head_dim 64, QK-RMSNorm + RoPE + per-head q gain).

Sharding: 8 cores = batch(4) x kv-head-pair(2). Each core handles one batch
element and 2 kv heads (= 8 query heads, 512 q dims), producing a partial
output projection against its 512 columns of Wproj. Host adds the two
partials per batch element.
"""

import numpy as np

import concourse.bass as bass
import concourse.mybir as mybir
import concourse.tile as tile
from concourse import bacc
from concourse.bass import ts
from concourse.masks import make_identity

F32 = mybir.dt.float32
F32R = mybir.dt.float32r

S = 2048          # sequence length
DIM = 1024        # model dim (contraction for qkv)
QM = 512          # q dims per core (8 heads x 64)
HD = 64           # head dim
NQH = 8           # local query heads
NKV = 2           # local kv heads
ND = DIM // 128   # 8 d-tiles
NST = S // 128    # 16 s-tiles
NSC = S // 512    # 4 s-chunks
NMT = QM // 128   # 4 q m-tiles
RMS_EPS = 1.1920928955078125e-07

TRACE = False
_CACHE = {}


def r(ap):
    return ap.bitcast(F32R)


def build_program(phases="abc", pbcast=True, do_rope=True, do_exp=True):
    nc = bacc.Bacc("TRN2", target_bir_lowering=False, debug=False)

    xT = nc.dram_tensor("xT", [DIM, S], F32R, kind="ExternalInput").ap()
    wqt = nc.dram_tensor("wqt", [DIM, QM], F32R, kind="ExternalInput").ap()
    wkvt = nc.dram_tensor("wkvt", [DIM, 256], F32R, kind="ExternalInput").ap()
    wpt = nc.dram_tensor("wpt", [QM, DIM], F32R, kind="ExternalInput").ap()
    gain8 = nc.dram_tensor("gain8", [128, NQH], F32, kind="ExternalInput").ap()
    cosd = nc.dram_tensor("cosd", [S, 32], F32, kind="ExternalInput").ap()
    sind = nc.dram_tensor("sind", [S, 32], F32, kind="ExternalInput").ap()
    maskd = nc.dram_tensor("maskd", [128, 4, 512], F32R, kind="ExternalInput").ap()
    out = nc.dram_tensor("out", [S, DIM], F32, kind="ExternalOutput").ap()

    with tile.TileContext(nc) as tc:
        from contextlib import ExitStack

        with ExitStack() as ctx:
            persist = ctx.enter_context(tc.tile_pool(name="persist", bufs=1))
            # persistent SBUF tensors
            qt_sb = persist.tile([128, NMT, S], F32R, tag="qt")     # Q^T  [qdim, s]
            kt_sb = persist.tile([128, S], F32R, tag="kt")          # K^T  [kdim, s]
            v_sb = persist.tile([128, NST, 130], F32R, tag="v")     # [V|1] per s-tile, 2 kv heads
            yt_sb = persist.tile([128, NMT, S], F32R, tag="yt")     # Y^T  [qdim, s]
            mask_sb = persist.tile([128, 4, 512], F32R, tag="mask")
            cos_sb = persist.tile([128, NST, 32], F32, tag="cos")
            sin_sb = persist.tile([128, NST, 32], F32, tag="sin")
            gain_sb = persist.tile([128, NQH], F32, tag="gain")
            ident = persist.tile([128, 128], F32, tag="ident")
            epsb = persist.tile([128, 1], F32, tag="epsb")

            make_identity(nc, ident[:])
            nc.vector.memset(epsb[:], RMS_EPS)
            nc.vector.memset(v_sb[:, :, 64:65].bitcast(mybir.dt.uint32), 0x3F800000)
            nc.vector.memset(v_sb[:, :, 129:130].bitcast(mybir.dt.uint32), 0x3F800000)
            nc.sync.dma_start(mask_sb[:], maskd)
            nc.sync.dma_start(cos_sb[:], cosd.rearrange("(st p) f -> p st f", p=128))
            nc.sync.dma_start(sin_sb[:], sind.rearrange("(st p) f -> p st f", p=128))
            nc.sync.dma_start(gain_sb[:], gain8)

            psum = ctx.enter_context(tc.tile_pool(name="psum", bufs=2, space="PSUM"))
            psum2 = ctx.enter_context(tc.tile_pool(name="psum2", bufs=2, space="PSUM"))

            # ---------------- Phase A: QKV projection + norm + rope -------------
            with ExitStack() as actx:
                wpool = actx.enter_context(tc.tile_pool(name="wpool", bufs=1))
                xpool = actx.enter_context(tc.tile_pool(name="xpool", bufs=1))
                atmp = actx.enter_context(tc.tile_pool(name="atmp", bufs=2))

                wqt_sb = wpool.tile([128, ND, QM], F32R, tag="wqt")
                wkvt_sb = wpool.tile([128, ND, 256], F32R, tag="wkvt")
                nc.sync.dma_start(
                    wqt_sb[:], wqt.rearrange("(dt p) m -> p dt m", p=128)
                )
                nc.sync.dma_start(
                    wkvt_sb[:], wkvt.rearrange("(dt p) m -> p dt m", p=128)
                )

                for sc in range(NSC):
                    x_tiles = []
                    for dt in range(ND):
                        xt = xpool.tile([128, 512], F32R, tag=f"x{dt}")
                        nc.sync.dma_start(xt[:], xT[ts(dt, 128), ts(sc, 512)])
                        x_tiles.append(xt)

                    for sl in range(4):  # s-tile within chunk
                        st = sc * 4 + sl

                        # --- Q: [128 s, 512 qdim] ---
                        qp = psum.tile([128, 512], F32, tag="mm512")
                        for dt in range(ND):
                            nc.tensor.matmul(
                                qp[:],
                                x_tiles[dt][:, ts(sl, 128)],
                                wqt_sb[:, dt, :],
                                start=(dt == 0),
                                stop=(dt == ND - 1),
                            )
                        # --- K|V: [128 s, 256] ---
                        kvp = psum.tile([128, 256], F32, tag="aux")
                        for dt in range(ND):
                            nc.tensor.matmul(
                                kvp[:],
                                x_tiles[dt][:, ts(sl, 128)],
                                wkvt_sb[:, dt, :],
                                start=(dt == 0),
                                stop=(dt == ND - 1),
                            )

                        # --- q rmsnorm (+gain/8) ---
                        q3 = qp.rearrange("p (h d) -> p h d", d=HD)
                        sq = atmp.tile([128, NQH, HD], F32, tag="sq")
                        nc.scalar.square(sq[:], q3)
                        ssum = atmp.tile([128, NQH], F32, tag="ssum")
                        nc.vector.reduce_sum(
                            ssum[:], sq[:], axis=mybir.AxisListType.X
                        )
                        rs = atmp.tile([128, NQH], F32, tag="rs")
                        nc.scalar.activation(
                            rs[:], ssum[:], mybir.ActivationFunctionType.Sqrt,
                            bias=epsb[:], scale=1.0 / HD,
                        )
                        rr = atmp.tile([128, NQH], F32, tag="rr")
                        nc.vector.reciprocal(rr[:], rs[:])
                        rr2 = atmp.tile([128, NQH], F32, tag="rr2")
                        nc.vector.tensor_mul(rr2[:], rr[:], gain_sb[:])
                        qn = atmp.tile([128, NQH, HD], F32, tag="qn")
                        nc.vector.tensor_tensor(
                            qn[:], q3,
                            rr2[:, :, None].to_broadcast((128, NQH, HD)),
                            mybir.AluOpType.mult,
                        )
                        # --- q rope ---
                        if do_rope:
                            cb = cos_sb[:, st, None, :].to_broadcast((128, NQH, 32))
                            sb = sin_sb[:, st, None, :].to_broadcast((128, NQH, 32))
                            qr = atmp.tile([128, NQH, HD], F32, tag="qr")
                            t1 = atmp.tile([128, NQH, 32], F32, tag="t1")
                            t2 = atmp.tile([128, NQH, 32], F32, tag="t2")
                            nc.vector.tensor_mul(t1[:], qn[:, :, 0:32], cb)
                            nc.vector.tensor_mul(t2[:], qn[:, :, 32:64], sb)
                            nc.vector.tensor_add(qr[:, :, 0:32], t1[:], t2[:])
                            t3 = atmp.tile([128, NQH, 32], F32, tag="t3")
                            t4 = atmp.tile([128, NQH, 32], F32, tag="t4")
                            nc.vector.tensor_mul(t3[:], qn[:, :, 0:32], sb)
                            nc.vector.tensor_mul(t4[:], qn[:, :, 32:64], cb)
                            nc.vector.tensor_tensor(
                                qr[:, :, 32:64], t4[:], t3[:], mybir.AluOpType.subtract
                            )
                        else:
                            qr = qn
                        # --- transpose q -> QT ---
                        qr2 = qr.rearrange("p h d -> p (h d)")
                        for mt in range(NMT):
                            tp = psum2.tile([128, 512], F32, tag="tpbp")
                            nc.tensor.transpose(
                                tp[:, 0:128], qr2[:, ts(mt, 128)], ident[:]
                            )
                            nc.vector.tensor_copy(
                                qt_sb[:, mt, ts(st, 128)], tp[:, 0:128]
                            )

                        # --- k rmsnorm ---
                        k3 = kvp[:, 0:128].rearrange("p (h d) -> p h d", d=HD)
                        sqk = atmp.tile([128, NKV, HD], F32, tag="sqk")
                        nc.scalar.square(sqk[:], k3)
                        ssk = atmp.tile([128, NKV], F32, tag="ssk")
                        nc.vector.reduce_sum(
                            ssk[:], sqk[:], axis=mybir.AxisListType.X
                        )
                        rsk = atmp.tile([128, NKV], F32, tag="rsk")
                        nc.scalar.activation(
                            rsk[:], ssk[:], mybir.ActivationFunctionType.Sqrt,
                            bias=epsb[:], scale=1.0 / HD,
                        )
                        rrk = atmp.tile([128, NKV], F32, tag="rrk")
                        nc.vector.reciprocal(rrk[:], rsk[:])
                        kn = atmp.tile([128, NKV, HD], F32, tag="kn")
                        nc.vector.tensor_tensor(
                            kn[:], k3,
                            rrk[:, :, None].to_broadcast((128, NKV, HD)),
                            mybir.AluOpType.mult,
                        )
                        # --- k rope ---
                        if do_rope:
                            cbk = cos_sb[:, st, None, :].to_broadcast((128, NKV, 32))
                            sbk = sin_sb[:, st, None, :].to_broadcast((128, NKV, 32))
                            kr = atmp.tile([128, NKV, HD], F32, tag="kr")
                            u1 = atmp.tile([128, NKV, 32], F32, tag="u1")
                            u2 = atmp.tile([128, NKV, 32], F32, tag="u2")
                            nc.vector.tensor_mul(u1[:], kn[:, :, 0:32], cbk)
                            nc.vector.tensor_mul(u2[:], kn[:, :, 32:64], sbk)
                            nc.vector.tensor_add(kr[:, :, 0:32], u1[:], u2[:])
                            u3 = atmp.tile([128, NKV, 32], F32, tag="u3")
                            u4 = atmp.tile([128, NKV, 32], F32, tag="u4")
                            nc.vector.tensor_mul(u3[:], kn[:, :, 0:32], sbk)
                            nc.vector.tensor_mul(u4[:], kn[:, :, 32:64], cbk)
                            nc.vector.tensor_tensor(
                                kr[:, :, 32:64], u4[:], u3[:], mybir.AluOpType.subtract
                            )
                        else:
                            kr = kn
                        # --- transpose k -> KT ---
                        kr2 = kr.rearrange("p h d -> p (h d)")
                        tpk = psum2.tile([128, 512], F32, tag="tpbp")
                        nc.tensor.transpose(tpk[:, 0:128], kr2, ident[:])
                        nc.vector.tensor_copy(kt_sb[:, ts(st, 128)], tpk[:, 0:128])

                        # --- V (stays [s, d]) ---
                        nc.vector.tensor_copy(v_sb[:, st, 0:64], kvp[:, 128:192])
                        nc.vector.tensor_copy(v_sb[:, st, 65:129], kvp[:, 192:256])

            # ---------------- Phase B: attention ------------------------------
            run_b = "b" in phases
            run_c = "c" in phases
            bpool = ctx.enter_context(tc.tile_pool(name="bpool", bufs=4))
            bsmall = ctx.enter_context(tc.tile_pool(name="bsmall", bufs=2))
            # Q heads are stored host-permuted as [0,4,1,5,2,6,3,7]: m-tile mt
            # holds head mt (kv group 0) on partitions 0-63 and head mt+4
            # (kv group 1) on partitions 64-127, so the scores matmul's two
            # operands always share the same base partition (= kv*64).
            for ic in range(NSC if run_b else 0):
                for h in range(NQH):
                    kv = h // 4
                    mt = h % 4
                    pr = kv * 64
                    qth = qt_sb[pr : pr + 64, mt, ts(ic, 512)]
                    yp = psum.tile([65, 512], F32, tag="yp")
                    njt = 4 * ic + 4
                    for jt in range(njt):
                        sp = psum.tile([128, 512], F32, tag="mm512")
                        nc.tensor.matmul(
                            sp[:],
                            kt_sb[kv * 64 : kv * 64 + 64, ts(jt, 128)],
                            qth,
                            start=True,
                            stop=True,
                        )
                        p = bpool.tile([128, 512], F32R, tag="p")
                        nc.scalar.activation(
                            p[:], sp[:],
                            mybir.ActivationFunctionType.Exp
                            if do_exp else mybir.ActivationFunctionType.Copy,
                        )
                        rbl = jt - 4 * ic
                        if rbl >= 0:
                            nc.vector.tensor_mul(p[:], p[:], mask_sb[:, rbl, :])
                        nc.tensor.matmul(
                            yp[:],
                            v_sb[:, jt, kv * 65 : kv * 65 + 65],
                            p[:],
                            start=(jt == 0),
                            stop=(jt == njt - 1),
                        )
                    # normalize: y / denom (denom = row 64)
                    rrow = bsmall.tile([1, 512], F32, tag="rrow")
                    nc.vector.reciprocal(rrow[:], yp[64:65, :])
                    if pbcast:
                        bs = bpool.tile([64, 512], F32, tag="bs")
                        nc.gpsimd.partition_broadcast(bs[:], rrow[:])
                        nc.vector.tensor_mul(
                            yt_sb[pr : pr + 64, mt, ts(ic, 512)], yp[0:64, :], bs[:]
                        )
                    else:
                        nc.vector.tensor_copy(
                            yt_sb[pr : pr + 64, mt, ts(ic, 512)], yp[0:64, :]
                        )

            if not run_c:
                # dummy: write qt to out so the program has an output
                dpool = ctx.enter_context(tc.tile_pool(name="dpool", bufs=2))
                for st in range(NST):
                    for nch in range(2):
                        db = dpool.tile([128, 512], F32, tag="db")
                        nc.vector.tensor_copy(
                            db[:], qt_sb[:, nch * 2, ts(st, 128), None]
                            .to_broadcast((128, 128, 4))
                            .rearrange("p a b -> p (a b)"),
                        )
                        nc.sync.dma_start(out[ts(st, 128), ts(nch, 512)], db[:])

            # ---------------- Phase C: output projection -----------------------
            with ExitStack() as cctx:
                cpool = cctx.enter_context(tc.tile_pool(name="cpool", bufs=1))
                opool = cctx.enter_context(tc.tile_pool(name="opool", bufs=3))
                wpt_sb = cpool.tile([128, NMT, DIM], F32R, tag="wpt")
                nc.sync.dma_start(
                    wpt_sb[:], wpt.rearrange("(mt p) n -> p mt n", p=128)
                )
                for st in range(NST):
                    for nch in range(2):
                        op = psum.tile([128, 512], F32, tag="mm512")
                        for mt in range(NMT):
                            nc.tensor.matmul(
                                op[:],
                                yt_sb[:, mt, ts(st, 128)],
                                wpt_sb[:, mt, ts(nch, 512)],
                                start=(mt == 0),
                                stop=(mt == NMT - 1),
                            )
                        ob = opool.tile([128, 512], F32, tag="ob")
                        nc.vector.tensor_copy(ob[:], op[:])
                        nc.sync.dma_start(out[ts(st, 128), ts(nch, 512)], ob[:])

    nc.compile()
    return nc


def _rope_tables():
    inv = (
        1.0 / (np.float32(10000.0) ** (np.arange(0, HD, 2, dtype=np.float32) / np.float32(HD)))
    ).astype(np.float32)
    freqs = np.arange(S, dtype=np.float32)[:, None] * inv[None, :]
    return np.cos(freqs).astype(np.float32), np.sin(freqs).astype(np.float32)


def _masks():
    j = np.arange(128)[:, None]
    i = np.arange(512)[None, :]
    m = np.zeros((128, 4, 512), np.float32)
    for rbl in range(4):
        m[:, rbl, :] = (j + 128 * rbl <= i).astype(np.float32)
    return m


HEAD_PERM = [0, 4, 1, 5, 2, 6, 3, 7]


def round_f32r(a):
    """Round fp32 to the FP32R format (8-bit exp, 11-bit mantissa stored in
    the top 20 bits), round-to-nearest-even, so the PE reads properly
    rounded operands instead of truncating."""
    u = np.ascontiguousarray(a, np.float32).view(np.uint32)
    u = u + 0x7FF + ((u >> 12) & 1)
    u &= np.uint32(0xFFFFF000)
    return u.view(np.float32)


def in_map_for_core(c, x, Wq, Wk, Wv, Wproj, q_gain, cos, sin, masks):
    b, hh = c // 2, c % 2
    g_sh = q_gain[8 * hh : 8 * hh + 8][HEAD_PERM]
    g8 = np.repeat((g_sh / 8.0)[None, :], 128, axis=0)
    wq_sh = (
        Wq[512 * hh : 512 * hh + 512, :].reshape(8, 64, DIM)[HEAD_PERM]
    ).reshape(512, DIM)
    return {
        "xT": round_f32r(x[b].T),
        "wqt": round_f32r(wq_sh.T),
        "wkvt": round_f32r(
            np.concatenate(
                [
                    Wk[128 * hh : 128 * hh + 128, :],
                    Wv[128 * hh : 128 * hh + 128, :],
                ],
                axis=0,
            ).T
        ),
        "wpt": round_f32r(
            Wproj[:, 512 * hh : 512 * hh + 512]
            .T.reshape(8, 64, DIM)[HEAD_PERM]
            .reshape(512, DIM)
        ),
        "gain8": np.ascontiguousarray(g8.astype(np.float32)),
        "cosd": cos,
        "sind": sin,
        "maskd": masks,
    }


def kernel(x, Wq, Wk, Wv, Wproj, q_gain):
    x = np.asarray(x, np.float32)
    Wq = np.asarray(Wq, np.float32)
    Wk = np.asarray(Wk, np.float32)
    Wv = np.asarray(Wv, np.float32)
    Wproj = np.asarray(Wproj, np.float32)
    q_gain = np.asarray(q_gain, np.float32)

    if "runner" not in _CACHE:
        _CACHE["runner"] = _build_runner(build_program())
    runner = _CACHE["runner"]

    cos, sin = _rope_tables()
    masks = _masks()

    in_maps = [
        in_map_for_core(c, x, Wq, Wk, Wv, Wproj, q_gain, cos, sin, masks)
        for c in range(8)
    ]

    results = runner.run(in_maps)

    out = np.empty((4, S, DIM), np.float32)
    for b in range(4):
        out[b] = results[2 * b]["out"] + results[2 * b + 1]["out"]
    return out


class _Runner:
    """Cached jit of the SPMD bass program on 8 axon TRN2 cores.

    Mirrors bass2jax.run_bass_via_pjrt but keeps the jitted callable (and
    the traced executable) alive so repeated calls don't retrace, which
    also makes back-to-back timing possible.
    """

    def __init__(self, nc, n_cores=8):
        import jax
        from jax.experimental.shard_map import shard_map
        from jax.sharding import Mesh, PartitionSpec

        from concourse import bass2jax

        bass2jax.install_neuronx_cc_hook()
        self.nc = nc
        self.n_cores = n_cores
        in_names: list[str] = []
        out_names: list[str] = []
        out_avals = []
        zero_outs = []
        part_name0 = nc.partition_id_tensor.name if nc.partition_id_tensor else None
        for alloc in nc.m.functions[0].allocations:
            if not isinstance(alloc, mybir.MemoryLocationSet):
                continue
            name = alloc.memorylocations[0].name
            if alloc.kind == "ExternalInput":
                if name != part_name0:
                    in_names.append(name)
            elif alloc.kind == "ExternalOutput":
                out_names.append(name)
                shape = tuple(alloc.tensor_shape)
                dtype = mybir.dt.np(alloc.dtype)
                out_avals.append(jax.core.ShapedArray(shape, dtype))
                zero_outs.append(np.zeros(shape, dtype))
        n_params = len(in_names)
        n_outs = len(out_avals)
        all_names = list(in_names) + list(out_names)
        part_name = nc.partition_id_tensor.name if nc.partition_id_tensor else None
        if part_name is not None:
            all_names.append(part_name)
        self.in_names = in_names
        self.out_names = out_names
        self.out_avals = out_avals
        self.zero_outs = zero_outs

        def _body(*args):
            operands = list(args)
            if part_name is not None:
                operands.append(bass2jax.partition_id_tensor())
            outs = bass2jax._bass_exec_p.bind(
                *operands,
                out_avals=tuple(out_avals),
                in_names=tuple(all_names),
                out_names=tuple(out_names),
                lowering_input_output_aliases=(),
                sim_require_finite=True,
                sim_require_nnan=True,
                nc=nc,
            )
            return tuple(outs)

        devices = jax.devices()[:n_cores]
        self.mesh = Mesh(np.asarray(devices), ("core",))
        in_specs = (PartitionSpec("core"),) * (n_params + n_outs)
        out_specs = (PartitionSpec("core"),) * n_outs
        donate = tuple(range(n_params, n_params + n_outs))
        self.sharded = jax.jit(
            shard_map(
                _body,
                mesh=self.mesh,
                in_specs=in_specs,
                out_specs=out_specs,
                check_rep=False,
            ),
            donate_argnums=donate,
            keep_unused=True,
        )

    def _concat_inputs(self, in_maps):
        return [
            np.concatenate([np.asarray(in_maps[c][n]) for c in range(self.n_cores)], axis=0)
            for n in self.in_names
        ]

    def _concat_zeros(self):
        return [
            np.zeros((self.n_cores * z.shape[0], *z.shape[1:]), z.dtype)
            for z in self.zero_outs
        ]

    def run(self, in_maps):
        out_arrs = self.sharded(*self._concat_inputs(in_maps), *self._concat_zeros())
        return [
            {
                n: np.asarray(out_arrs[i]).reshape(
                    self.n_cores, *self.out_avals[i].shape
                )[c]
                for i, n in enumerate(self.out_names)
            }
            for c in range(self.n_cores)
        ]

    def bench(self, in_maps, iters=20):
        """Average wall time per execution with device-resident inputs."""
        import time

        import jax
        from jax.sharding import NamedSharding, PartitionSpec

        sh = NamedSharding(self.mesh, PartitionSpec("core"))
        ins_dev = [jax.device_put(a, sh) for a in self._concat_inputs(in_maps)]
        zero_sets = [
            [jax.device_put(z, sh) for z in self._concat_zeros()]
            for _ in range(iters + 2)
        ]
        # warmup
        for i in range(2):
            r = self.sharded(*ins_dev, *zero_sets[i])
        jax.block_until_ready(r)
        t0 = time.time()
        outs = [self.sharded(*ins_dev, *zero_sets[2 + i]) for i in range(iters)]
        jax.block_until_ready(outs)
        t1 = time.time()
        return (t1 - t0) / iters


def _build_runner(nc):
    return _Runner(nc)

